# revision 69
# baseline (speedup 1.0000x reference)
"""Trainium2 Bass kernel for DFine multi-head attention.

Problem: B=2, S=2048, D=1024, H=16 heads, HD=64.
Sharding over 8 cores: core c handles batch b=c//4 and head-group g=c%4
(4 heads). Each core computes its heads' attention and a partial
out-projection [2048, 1024]; the host sums the 4 partials per batch and
adds the output bias. h = x + pos is folded on the host (f32) so the
kernel receives hT and xT directly in bf16.

All matmuls run in bf16 (f32 PSUM accumulation, rel err ~8e-3 vs the
f32 reference). attnV is computed transposed — out[t,he] with the full
128 output partitions — which halves its PE cost vs the [he,t] form;
the [t,he]->[he,t] flip for the out-projection rides the XBAR DMA
transpose instead of the PE. Emission is a fine-grained weave so the
scalar engine's exp stream (the co-bottleneck) never starves.
"""

import sys
import numpy as np

if "/opt/trn_rl_repo" not in sys.path:
    sys.path.insert(0, "/opt/trn_rl_repo")

B, S, D, H, HD = 2, 2048, 1024, 16, 64
G = 4          # heads per core
E = G * HD     # 256 per-core head width
T = S          # tokens
KC = 8         # contraction chunks of 128 over D
TB = 512       # t-block (scores moving free dim)
NT = T // TB   # 4
NS = T // 128  # 16 s-chunks
HA = HD + 1    # head width + denominator column
SCALE = HD ** -0.5

_PROGRAM = None


def _build_program(reps=1):
    import concourse.bacc as bacc
    import concourse.tile as tile
    from concourse import mybir

    f32 = mybir.dt.float32
    bf16 = mybir.dt.bfloat16

    nc = bacc.Bacc("TRN2", target_bir_lowering=False, debug=False)

    hT_d = nc.declare_dram_parameter("hT", [D, T], bf16, isOutput=False)
    pv_d = nc.declare_dram_parameter("pv", [T, E], bf16, isOutput=False)
    wq_d = nc.declare_dram_parameter("wq", [D, E], bf16, isOutput=False)
    wk_d = nc.declare_dram_parameter("wk", [D, E], bf16, isOutput=False)
    wv_d = nc.declare_dram_parameter("wv", [D, E], bf16, isOutput=False)
    wo_d = nc.declare_dram_parameter("wo", [E, D], bf16, isOutput=False)
    bq_d = nc.declare_dram_parameter("bq", [2, 128, 1], f32, isOutput=False)
    bk_d = nc.declare_dram_parameter("bk", [2, 128, 1], f32, isOutput=False)
    out_d = nc.declare_dram_parameter("out", [T, D], bf16, isOutput=True)

    with tile.TileContext(nc) as tc:
        for rep in range(reps):
            _build_body(nc, tc, mybir, rep,
                        (hT_d, pv_d, wq_d, wk_d, wv_d, wo_d, bq_d, bk_d,
                         out_d))

    nc.compile()
    return nc


def _build_body(nc, tc, mybir, rep, drams):
    from contextlib import ExitStack

    f32 = mybir.dt.float32
    bf16 = mybir.dt.bfloat16
    Exp = mybir.ActivationFunctionType.Exp
    add = mybir.AluOpType.add
    (hT_d, pv_d, wq_d, wk_d, wv_d, wo_d, bq_d, bk_d, out_d) = drams
    R = f"r{rep}_"

    octx = ExitStack()
    wpool = octx.enter_context(tc.tile_pool(name=f"{R}wpool", bufs=1))
    ps = octx.enter_context(tc.tile_pool(name=f"{R}ps", bufs=1,
                                         space="PSUM"))

    # ---- persistent tiles ----
    wq_t = wpool.tile([128, KC, E], bf16, name=f"{R}wq_t")
    wk_t = wpool.tile([128, KC, E], bf16, name=f"{R}wk_t")
    wv_t = wpool.tile([128, KC, E], bf16, name=f"{R}wv_t")
    wo_t = wpool.tile([128, 2, D], bf16, name=f"{R}wo_t")
    bq_t = wpool.tile([128, 2, 1], f32, name=f"{R}bq_t")
    bk_t = wpool.tile([128, 2, 1], f32, name=f"{R}bk_t")
    hT_t = wpool.tile([128, KC, T], bf16, name=f"{R}hT_t")
    pv_t = wpool.tile([128, NS, E], bf16, name=f"{R}pv_t")

    qT = [wpool.tile([128, T], bf16, name=f"{R}qT{p}") for p in range(2)]
    kT = [wpool.tile([128, T], bf16, name=f"{R}kT{p}") for p in range(2)]
    v_aug = wpool.tile([128, NS, G, HA], bf16, name=f"{R}v_aug")

    # ---- DMAs: q/k weights first (phase-A critical), then inputs ----
    wup = wpool.tile([128, 512], bf16, name=f"{R}wup")
    nc.vector.memset(wup[:], 0.0)
    nc.vector.memset(v_aug[:, :, :, HD:HA], 1.0)

    # everything on the one sync queue so DMA_ENGINES order is exactly
    # this: q/k weights + biases, hT half-0 (q/k critical path), xT
    # half-0 + wv (v_proj), hT half-1, xT half-1, wo (out_proj only).
    HT = T // 2

    def _in_chunks(dst, src, hf):
        for k in range(KC):
            nc.sync.dma_start(
                dst[:, k, hf * HT:(hf + 1) * HT],
                src[k * 128:(k + 1) * 128, hf * HT:(hf + 1) * HT])

    nc.sync.dma_start(wk_t[:], wk_d[:].rearrange("(c p) e -> p c e", p=128))
    nc.sync.dma_start(
        wq_t[:, :, 0:128],
        wq_d[:, 0:128].rearrange("(c p) e -> p c e", p=128))
    _in_chunks(hT_t, hT_d, 0)
    nc.sync.dma_start(bq_t[:], bq_d[:].rearrange("c p o -> p c o"))
    nc.sync.dma_start(bk_t[:], bk_d[:].rearrange("c p o -> p c o"))
    _in_chunks(hT_t, hT_d, 1)
    nc.sync.dma_start(
        wq_t[:, :, 128:256],
        wq_d[:, 128:256].rearrange("(c p) e -> p c e", p=128))
    nc.sync.dma_start(wv_t[:], wv_d[:].rearrange("(c p) e -> p c e", p=128))
    nc.sync.dma_start(pv_t[:], pv_d[:].rearrange("(s p) e -> p s e", p=128))
    nc.sync.dma_start(wo_t[:], wo_d[:].rearrange("(c p) d -> p c d", p=128))

    # identity for the PE-transpose tail of the last t-block
    from concourse.masks import make_identity
    ident = wpool.tile([128, 128], bf16, name=f"{R}ident")
    make_identity(nc, ident)

    # p-state warmup: keep the PE busy from t~0 so it reaches full clock
    # before the projection stream starts (dummy results are discarded).
    for w in range(8):
        wps = ps.tile([1, 512], f32, name=f"{R}wps_{w}", tag="sc0")
        nc.tensor.matmul(wps[:], wup[:, 0:1], wup[:], start=True, stop=True)

    # ---- emission units ----------------------------------------------

    # q/k pair-0 projection wave: two 8-step psum groups, k-outer so each
    # k-step consumes its hT chunk as the DMA lands. Waves are ordered so
    # the k-side (which gates the scores' stationaries) lands first.
    def qk_p0_wave(groups, tags, act_bias=False):
        pss = {}
        for (nm, tb), tag in zip(groups, tags):
            pss[(nm, tb)] = ps.tile([128, TB], f32,
                                    name=f"{R}{nm}ps0_{tb}", tag=tag)
        for k in range(KC):
            for nm, tb in groups:
                w_t = wq_t if nm == "q" else wk_t
                nc.tensor.matmul(
                    pss[(nm, tb)][:], w_t[:, k, 0:128],
                    hT_t[:, k, tb * TB:(tb + 1) * TB],
                    start=(k == 0), stop=(k == KC - 1))
        for i, (nm, tb) in enumerate(groups):
            dst = qT if nm == "q" else kT
            b_t = bq_t if nm == "q" else bk_t
            d = dst[0][:, tb * TB:(tb + 1) * TB]
            if act_bias and i % 2 == 0:
                nc.scalar.activation(
                    d, pss[(nm, tb)][:],
                    mybir.ActivationFunctionType.Identity,
                    bias=b_t[:, 0, 0:1])
            else:
                nc.vector.tensor_scalar_add(d, pss[(nm, tb)][:],
                                            b_t[:, 0, 0:1])

    # k pair-1 projection for one t-block (rides the exp windows of the
    # first two pairs on the attnV psum tags, which are idle until then)
    def kp1(tb):
        psx = ps.tile([128, TB], f32, name=f"{R}kps1_{tb}",
                      tag=f"av{tb % 2}")
        for k in range(KC):
            nc.tensor.matmul(
                psx[:], wk_t[:, k, 128:256],
                hT_t[:, k, tb * TB:(tb + 1) * TB],
                start=(k == 0), stop=(k == KC - 1))
        nc.vector.tensor_scalar_add(
            kT[1][:, tb * TB:(tb + 1) * TB], psx[:], bk_t[:, 1, 0:1])

    # q pair-1 projection for one t-block
    def qp1(tb):
        psx = ps.tile([128, TB], f32, name=f"{R}qps1_{tb}",
                      tag=f"qk{tb % 2}")
        for k in range(KC):
            nc.tensor.matmul(
                psx[:], wq_t[:, k, 128:256],
                hT_t[:, k, tb * TB:(tb + 1) * TB],
                start=(k == 0), stop=(k == KC - 1))
        nc.vector.tensor_scalar_add(
            qT[1][:, tb * TB:(tb + 1) * TB], psx[:], bq_t[:, 1, 0:1])

    # v projection for two s-chunks of one head-pair: v = h*Wv - pos*Wv
    # (the pos*Wv - bv correction is precomputed on the host so the x
    # tensor never ships; hT doubles as the stationary)
    sub = mybir.AluOpType.subtract
    def v_u(u, hp):
        c0 = hp * 128
        for si in (2 * u, 2 * u + 1):
            psx = ps.tile([128, 128], f32, name=f"{R}vp_{si}_{hp}",
                          tag=f"qk{si % 2}")
            for k in range(KC):
                nc.tensor.matmul(
                    psx[:], hT_t[:, k, si * 128:(si + 1) * 128],
                    wv_t[:, k, c0:c0 + 128],
                    start=(k == 0), stop=(k == KC - 1))
            nc.vector.tensor_tensor(
                v_aug[:, si, 2 * hp:2 * hp + 2, 0:HD],
                psx[:].rearrange("p (g e) -> p g e", g=2),
                pv_t[:, si, c0:c0 + 128].rearrange("p (g e) -> p g e", g=2),
                op=sub)

    cctx = ExitStack()
    expool = cctx.enter_context(tc.tile_pool(name=f"{R}expool", bufs=2))
    apool = cctx.enter_context(tc.tile_pool(name=f"{R}apool", bufs=2))
    opool = cctx.enter_context(tc.tile_pool(name=f"{R}opool", bufs=3))
    rpool = cctx.enter_context(tc.tile_pool(name=f"{R}rpool", bufs=2))

    ex_tiles = {}
    att_tiles = {}
    a2_tiles = {}

    # scores + exp for two s-chunks of pair (tb, p)
    def sc_unit(tb, p, c):
        t0 = tb * TB
        if p == 0 and c == 0:
            att_tiles[tb] = apool.tile([128, NT, E], bf16,
                                       name=f"{R}att_{tb}", tag="attnT")
        for si in (2 * c, 2 * c + 1):
            scp = ps.tile([128, 2, TB], f32,
                          name=f"{R}sc_{tb}_{p}_{si}", tag=f"sc{si % 2}")
            for h in range(2):
                nc.tensor.matmul(
                    scp[:, h, :],
                    kT[p][h * 64:(h + 1) * 64, si * 128:(si + 1) * 128],
                    qT[p][h * 64:(h + 1) * 64, t0:t0 + TB],
                    start=True, stop=True)
            ex = expool.tile([128, 2, TB], bf16,
                             name=f"{R}ex_{tb}_{p}_{si}", tag=f"ex{si}")
            nc.scalar.activation(ex[:], scp[:], Exp)
            ex_tiles[(tb, p, si)] = ex

    def _normalize(av, att, tb, p, tcc, on_act=False):
        rec = rpool.tile([128, 2], f32, name=f"{R}rc_{tb}_{p}_{tcc}",
                         tag=f"rec{tcc % 2}")
        with nc.allow_low_precision(reason="softmax denominator"):
            for h in range(2):
                nc.vector.reciprocal(rec[:, h:h + 1],
                                     av[:, h * HA + HD:h * HA + HD + 1])
        for h in range(2):
            d = att[:, tcc, p * 128 + h * 64:p * 128 + (h + 1) * 64]
            if on_act:
                nc.scalar.activation(d, av[:, h * HA:h * HA + HD],
                                     mybir.ActivationFunctionType.Copy,
                                     scale=rec[:, h:h + 1])
            else:
                nc.vector.tensor_scalar_mul(d, av[:, h * HA:h * HA + HD],
                                            rec[:, h:h + 1])

    def _transpose(tb, tcc, hc):
        if tb not in a2_tiles:
            a2_tiles[tb] = apool.tile([128, 2, TB], bf16,
                                      name=f"{R}a2_{tb}", tag="attn2")
        nc.sync.dma_start_transpose(
            a2_tiles[tb][:, hc, tcc * 128:(tcc + 1) * 128],
            att_tiles[tb][:, tcc, hc * 128:(hc + 1) * 128])

    # PE-based transpose for the last t-block's tail (no HWDGE roundtrip)
    def tp_unit(tb, tcc):
        if tb not in a2_tiles:
            a2_tiles[tb] = apool.tile([128, 2, TB], bf16,
                                      name=f"{R}a2_{tb}", tag="attn2")
        for hc in range(2):
            tps = ps.tile([128, 128], bf16, name=f"{R}tp_{tcc}_{hc}",
                          tag=f"sc{hc}")
            nc.tensor.transpose(
                tps[:], att_tiles[tb][:, tcc, hc * 128:(hc + 1) * 128],
                ident[:])
            nc.vector.tensor_copy(
                a2_tiles[tb][:, hc, tcc * 128:(tcc + 1) * 128], tps[:])

    # transposed attnV for one t-chunk of pair (tb, p); with_t emits the
    # XBAR transposes for this chunk right after the normalize
    def av_unit(tb, p, tcc, with_t=False, on_act=False, tag=None):
        att = att_tiles[tb]
        av = ps.tile([128, 2 * HA], f32, name=f"{R}av_{tb}_{p}_{tcc}",
                     tag=tag or f"av{tcc % 2}")
        for h in range(2):
            for si in range(NS):
                nc.tensor.matmul(
                    av[:, h * HA:(h + 1) * HA],
                    ex_tiles[(tb, p, si)][:, h, tcc * 128:(tcc + 1) * 128],
                    v_aug[:, si, p * 2 + h, :],
                    start=(si == 0), stop=(si == NS - 1))
        _normalize(av, att, tb, p, tcc, on_act=on_act)
        if with_t:
            _transpose(tb, tcc, 0)
            _transpose(tb, tcc, 1)

    # out-projection for one 128-token chunk of t-block tb. The last
    # t-block's psum->sbuf copies ride the by-then-idle ACT engine and
    # the store is split per half so it overlaps the second copy.
    def fin_unit(tb, ts):
        a2 = a2_tiles[tb]
        tsl = tb * TB + ts * 128
        last = tb == NT - 1
        osb = opool.tile([128, D], bf16, name=f"{R}osb_{tb}_{ts}",
                         tag="osb")
        for dc in range(2):
            psx = ps.tile([128, 512], f32, name=f"{R}op_{tb}_{ts}_{dc}",
                          tag=f"qk{dc}")
            for hc in range(2):
                nc.tensor.matmul(
                    psx[:], a2[:, hc, ts * 128:(ts + 1) * 128],
                    wo_t[:, hc, dc * 512:(dc + 1) * 512],
                    start=(hc == 0), stop=(hc == 1))
            dst = osb[:, dc * 512:(dc + 1) * 512]
            if last:
                nc.scalar.activation(dst, psx[:],
                                     mybir.ActivationFunctionType.Copy)
                nc.sync.dma_start(
                    out_d[tsl:tsl + 128, dc * 512:(dc + 1) * 512], dst)
            else:
                nc.vector.tensor_copy(dst, psx[:])
        if not last:
            nc.sync.dma_start(out_d[tsl:tsl + 128, :], osb[:])

    # ---- the weave ----------------------------------------------------
    SC = sc_unit
    AV = av_unit

    # prologue + pair (0,0): scores ride the hT DMA halves; k pair-1 and
    # v fill the DMA-bound stretch and the first exp window
    qk_p0_wave([("k", 0), ("q", 0)], ["av0", "av1"], act_bias=True)
    qk_p0_wave([("k", 1), ("q", 1)], ["qk0", "qk1"], act_bias=True)
    SC(0, 0, 0); SC(0, 0, 1)
    kp1(0); kp1(1)
    qk_p0_wave([("k", 2), ("k", 3)], ["av0", "av1"])
    SC(0, 0, 2); SC(0, 0, 3)
    qk_p0_wave([("q", 2), ("q", 3)], ["qk0", "qk1"])
    SC(0, 0, 4); SC(0, 0, 5)
    SC(0, 0, 6); SC(0, 0, 7)
    qp1(0)
    kp1(2); kp1(3)

    # pair (0,1): v heads 0-1 + attnV(0,0)
    SC(0, 1, 0); SC(0, 1, 1)
    v_u(0, 0); v_u(1, 0)
    SC(0, 1, 2); SC(0, 1, 3)
    v_u(2, 0); v_u(3, 0)
    SC(0, 1, 4); SC(0, 1, 5)
    v_u(4, 0); v_u(5, 0)
    SC(0, 1, 6); SC(0, 1, 7)
    v_u(6, 0); v_u(7, 0)
    AV(0, 0, 0); AV(0, 0, 1); AV(0, 0, 2); AV(0, 0, 3)

    # pair (1,0): v heads 2-3 + attnV(0,1) + transposes
    v_u(0, 1); v_u(1, 1); v_u(2, 1); v_u(3, 1)
    SC(1, 0, 0); SC(1, 0, 1)
    v_u(4, 1); v_u(5, 1); v_u(6, 1); v_u(7, 1)
    SC(1, 0, 2); SC(1, 0, 3)
    AV(0, 1, 0, with_t=True)
    SC(1, 0, 4); SC(1, 0, 5)
    AV(0, 1, 1, with_t=True); AV(0, 1, 2, with_t=True)
    SC(1, 0, 6); SC(1, 0, 7)
    AV(0, 1, 3, with_t=True)

    # pairs (tb, 0): attnV(tb-1,1) + transposes + out-proj; (tb, 1):
    # attnV(tb,0) + out-proj quarters
    def pair_a(tb, fins):
        SC(tb, 0, 0); SC(tb, 0, 1)
        AV(tb - 1, 1, 0, with_t=True)
        SC(tb, 0, 2); SC(tb, 0, 3)
        AV(tb - 1, 1, 1, with_t=True)
        fins and fins[0]()
        SC(tb, 0, 4); SC(tb, 0, 5)
        AV(tb - 1, 1, 2, with_t=True)
        fins and fins[1]()
        SC(tb, 0, 6); SC(tb, 0, 7)
        AV(tb - 1, 1, 3, with_t=True)

    def pair_b(tb, fins):
        qp1(tb)
        SC(tb, 1, 0); SC(tb, 1, 1)
        AV(tb, 0, 0)
        fins and fins[0]()
        SC(tb, 1, 2); SC(tb, 1, 3)
        AV(tb, 0, 1)
        fins and fins[1]()
        SC(tb, 1, 4); SC(tb, 1, 5)
        AV(tb, 0, 2)
        fins and len(fins) > 2 and fins[2]()
        SC(tb, 1, 6); SC(tb, 1, 7)
        AV(tb, 0, 3)
        fins and len(fins) > 3 and fins[3]()

    def F(tb, ts):
        return lambda: fin_unit(tb, ts)

    pair_b(1, [F(0, 0), F(0, 1), F(0, 2), F(0, 3)])
    pair_a(2, [F(1, 0), F(1, 1)])
    pair_b(2, [F(1, 2), F(1, 3)])
    pair_a(3, [F(2, 0), F(2, 1)])
    pair_b(3, [F(2, 2), F(2, 3)])

    # tail: last pair's attnV + PE transposes + out-projection, chained
    # per t-chunk so the first store leaves as early as possible
    tb = NT - 1
    AV(tb, 1, 0); AV(tb, 1, 1)
    tp_unit(tb, 0)
    fin_unit(tb, 0)
    AV(tb, 1, 2)
    tp_unit(tb, 1)
    fin_unit(tb, 1)
    AV(tb, 1, 3)
    tp_unit(tb, 2)
    fin_unit(tb, 2)
    tp_unit(tb, 3)
    fin_unit(tb, 3)

    cctx.close()
    octx.close()


def _get_program(reps=1):
    global _PROGRAM
    if _PROGRAM is None:
        _PROGRAM = {}
    if reps not in _PROGRAM:
        _PROGRAM[reps] = _build_program(reps)
    return _PROGRAM[reps]


def _shard_inputs(inputs):
    """Build the 8 per-core input maps from the full-problem inputs."""
    import ml_dtypes
    bf16 = ml_dtypes.bfloat16

    hs = np.asarray(inputs["hidden_states"], np.float32)
    pe = np.asarray(inputs["position_embeddings"], np.float32)
    Wq = np.asarray(inputs["Wq"], np.float32).reshape(D, H * HD)
    Wk = np.asarray(inputs["Wk"], np.float32).reshape(D, H * HD)
    Wv = np.asarray(inputs["Wv"], np.float32).reshape(D, H * HD)
    Wo = np.asarray(inputs["Wo"], np.float32)
    bq = np.asarray(inputs["bq"], np.float32).reshape(H * HD)
    bk = np.asarray(inputs["bk"], np.float32).reshape(H * HD)
    bv = np.asarray(inputs["bv"], np.float32).reshape(H * HD)

    h = hs + pe
    hT = [np.ascontiguousarray(h[b].T).astype(bf16) for b in range(B)]

    in_maps = []
    for c in range(8):
        b, g = divmod(c, G)
        sel = slice(g * E, (g + 1) * E)
        pv = pe[b] @ Wv[:, sel] - bv[sel][None, :]
        in_maps.append({
            "hT": hT[b],
            "pv": np.ascontiguousarray(pv).astype(bf16),
            "wq": np.ascontiguousarray(
                Wq[:, sel] * np.float32(SCALE)).astype(bf16),
            "wk": np.ascontiguousarray(Wk[:, sel]).astype(bf16),
            "wv": np.ascontiguousarray(Wv[:, sel]).astype(bf16),
            "wo": np.ascontiguousarray(Wo[sel, :]).astype(bf16),
            "bq": (bq[sel] * np.float32(SCALE)).reshape(2, 128, 1).copy(),
            "bk": bk[sel].reshape(2, 128, 1).copy(),
        })
    return in_maps


def _gather_outputs(results, inputs):
    bo = np.asarray(inputs["bo"], np.float32)
    out = np.empty((B, S, D), np.float32)
    for b in range(B):
        acc = results[4 * b]["out"].astype(np.float32)
        for g in range(1, G):
            acc += results[4 * b + g]["out"].astype(np.float32)
        out[b] = acc + bo[None, :]
    return out


def kernel(**inputs):
    from concourse.bass_utils import run_bass_kernel_spmd

    nc = _get_program()
    in_maps = _shard_inputs(inputs)
    res = run_bass_kernel_spmd(nc, in_maps, list(range(8)))
    return _gather_outputs(res.results, inputs)
